# revision 17
# baseline (speedup 1.0000x reference)
"""Trainium2 Bass kernel for nn_MemoryAttention (causal single-head attention
with SiLU-gated output projection), sequence-parallel across 8 NeuronCores.

Strategy (per core c):
  - q rows owned: 4 slots of 256 rows: tile t = c + 8*s (strided assignment
    balances causal work; every core runs an identical instruction stream).
  - K/Q projections run in fp8 e4m3 with DoubleRow (2 MACs/cycle): host ships
    x*16 and wq/wk*256 in fp8; KT and QT are stored fp8 at 64x true scale and
    the exp() scale absorbs the 4096x logit scaling.  V path stays bf16.
  - Each core projects KT(fp8)/V(bf16) for its own rows, AllGathers them
    (kt first, then v, per slot-group) and keeps a small SBUF-resident
    duplicated prefix (b_dup kv blocks) to cover the collective's latency.
  - Masks for the diagonal visits are DMA'd once into SBUF at startup.
  - Per (slot, kv-block) visit: LT[kv, q] = K @ QT accumulated in PSUM,
    PT = exp(LT*2^-17) (* mask for the last 16 visits of each slot), then PT
    q-chunks become the stationary operand for both H[q, d] += P @ V (N=512)
    and rowsums += P @ 1 (N=1, shares the LDW).  kv blocks are fetched in
    pairs (one DMA each for kt/v) to halve DMA-queue occupancy.
  - Slot epilogue: H / sums, SiLU, PE-transpose of G, output projection with
    G^T chunks stationary -> O[q, d] written directly.
"""

import numpy as np
import ml_dtypes

import concourse.bass as bass
import concourse.tile as tile
from concourse import bacc, mybir
from concourse.bass_utils import run_bass_kernel_spmd
from concourse.masks import make_identity

P = 128
D = 1024
SEQ = 8192
NCORES = 8
NSLOTS = 4
QT_COLS = NSLOTS * 256
B_DUP = 4
N_MASKED = NSLOTS * 16  # visits with j >= 16*s need a mask on some core

F32 = mybir.dt.float32
BF16 = mybir.dt.bfloat16
FP8 = mybir.dt.float8e4
AF = mybir.ActivationFunctionType
DR = mybir.MatmulPerfMode.DoubleRow

X_SCALE = 16.0
W_SCALE = 256.0
KQ_SCALE = 64.0  # stored scale of kt/qt relative to true values
# psum kt/qt come out at X_SCALE*W_SCALE; cast down to KQ_SCALE
CAST_SCALE = KQ_SCALE / (X_SCALE * W_SCALE)
# logits psum = KQ_SCALE^2 * (q.k); want exp((q.k) * 2^-5)
EXP_SCALE = 2.0**-5 / (KQ_SCALE * KQ_SCALE)


def build_kernel(b_dup=B_DUP):
    assert b_dup % 4 == 0, "kt_proj writes the dup prefix in 512-column chunks"
    nc = bacc.Bacc(None, target_bir_lowering=False, num_devices=NCORES)

    # all inputs ship pre-arranged to the SBUF [P, 8, cols] layout so every
    # load is one contiguous-per-partition DMA (descriptor-light)
    xq8_ext = nc.declare_dram_parameter("xq8", [P, 8, QT_COLS], FP8, isOutput=False)
    xq_ext = nc.declare_dram_parameter("xq", [P, 8, QT_COLS], BF16, isOutput=False)
    if b_dup:
        xd_ext = nc.declare_dram_parameter("xd", [P, 8, b_dup * P], BF16, isOutput=False)
    wq8_ext = nc.declare_dram_parameter("wq8", [P, 8, D], FP8, isOutput=False)
    wk8_ext = nc.declare_dram_parameter("wk8", [P, 8, D], FP8, isOutput=False)
    wv1_ext = nc.declare_dram_parameter("wv1", [P, 8, D], BF16, isOutput=False)
    wv2_ext = nc.declare_dram_parameter("wv2", [P, 8, D], BF16, isOutput=False)
    mask_ext = nc.declare_dram_parameter("masks", [P, N_MASKED, 256], FP8, isOutput=False)
    o_ext = nc.declare_dram_parameter("o", [NSLOTS, 2, P, D], BF16, isOutput=True)

    # blocked kv payloads: [slot-in-grp][half][128 part][8][128]
    ktloc = nc.dram_tensor("ktloc", [2, 2, 2, P, 8, P], FP8)
    vloc = nc.dram_tensor("vloc", [2, 2, 2, P, 8, P], BF16)
    ktg = nc.dram_tensor("ktg", [NSLOTS, NCORES, 2, P, 8, P], FP8, addr_space="Shared")
    vg = nc.dram_tensor("vg", [NSLOTS, NCORES, 2, P, 8, P], BF16, addr_space="Shared")

    def wload(nc, pool, ext, tag, dtype=BF16, q=None):
        t = pool.tile([P, 8, D], dtype, tag=tag, name=tag)
        (q or nc.sync).dma_start(out=t, in_=ext[:])
        return t

    with tile.TileContext(nc) as tc:
        singles_ctx = tc.tile_pool(name="singles", bufs=1)
        singles = singles_ctx.__enter__()

        qt8_sb = singles.tile([P, 8, QT_COLS], FP8)
        masks_sb = singles.tile([P, N_MASKED, 256], FP8)
        if b_dup:
            dup_kt = singles.tile([P, b_dup, 8, P], FP8)
            dup_v = singles.tile([P, b_dup, 8, P], BF16)

        with (
            tc.tile_pool(name="projw", bufs=1) as projw,
            tc.tile_pool(name="projout", bufs=6) as projout,
            tc.tile_pool(name="ppsum", bufs=6, space="PSUM") as ppsum,
        ):
            # critical-path loads on the sync queue, in consumption order
            wk8_bf = wload(nc, projw, wk8_ext, "wk8", FP8)
            xq8_bf = projw.tile([P, 8, QT_COLS], FP8, tag="xq8", name="xq8")
            nc.sync.dma_start(out=xq8_bf, in_=xq8_ext[:])
            xq_bf = projw.tile([P, 8, QT_COLS], BF16, tag="xq", name="xq")
            nc.sync.dma_start(out=xq_bf, in_=xq_ext[:])
            wv1_bf = wload(nc, projw, wv1_ext, "wv1")
            # bulk loads on the scalar queue; the gpsimd queue carries ONLY
            # the collective doorbells so the gathers launch asap
            wq8_bf = wload(nc, projw, wq8_ext, "wq8", FP8, q=nc.scalar)

            ones_sb = singles.tile([P, 1], BF16)
            nc.vector.memset(ones_sb, 1.0)
            zcol_sb = singles.tile([1, P], BF16)
            nc.vector.memset(zcol_sb, 0.0)
            zrow_sb = singles.tile([1, 512], BF16)
            nc.vector.memset(zrow_sb, 0.0)
            ident_sb = singles.tile([P, P], BF16)
            make_identity(nc, ident_sb)

            # warm the PE clock gate during the input-load window: ~45 dummy
            # matmuls keep HAM busy so the projections start at full clock
            with tc.tile_pool(name="warm", bufs=1, space="PSUM") as warmp:
                wacc = warmp.tile([P, 512], F32, tag="warm", name="warm")
                for _ in range(24):
                    nc.tensor.matmul(
                        wacc, lhsT=zcol_sb, rhs=zrow_sb, start=True, stop=True
                    )

            def kt_proj(dst_sb, dst_dram, w8, src8, col0, col1):
                # KT payload per block: [p(dout), m, c] fp8 at KQ_SCALE
                # fp8 DoubleRow: contraction pairs of d-subtiles.  CASTs land
                # in SBUF; own groups then ship 4 contiguous per-block DMAs.
                chunks = list(range(col0 // 512, col1 // 512))
                for m in range(8):
                    accs = [
                        ppsum.tile([P, 512], F32, tag="proj", name=f"ktp{i}")
                        for i in range(len(chunks))
                    ]
                    for ss in range(0, 8, 2):
                        for i, n in enumerate(chunks):
                            nc.tensor.matmul(
                                accs[i],
                                lhsT=w8[:, ss : ss + 2, m * P : (m + 1) * P],
                                rhs=src8[:, ss : ss + 2, n * 512 : (n + 1) * 512],
                                start=(ss == 0),
                                stop=(ss == 6),
                                perf_mode=DR,
                            )
                    for i, n in enumerate(chunks):
                        nc.scalar.activation(
                            out=dst_sb[:, i * 4 : i * 4 + 4, m, :],
                            in_=accs[i].rearrange("p (b c) -> p b c", b=4),
                            func=AF.Copy,
                            scale=CAST_SCALE,
                        )
                if dst_dram is not None:
                    for b in range(4):
                        nc.sync.dma_start(
                            out=dst_dram[b // 2, b % 2], in_=dst_sb[:, b]
                        )

            def v_proj(dst_sb, dst_dram, wv, src, col0, col1):
                for blk in range(col0 // P, col1 // P):
                    v_out = projout.tile([P, 1024], BF16, tag="v_out", name="vo")
                    accs = [
                        ppsum.tile([P, 512], F32, tag="proj", name=f"vp{h2}")
                        for h2 in range(2)
                    ]
                    for sub in range(8):
                        for h2 in range(2):
                            nc.tensor.matmul(
                                accs[h2],
                                lhsT=src[:, sub, blk * P : (blk + 1) * P],
                                rhs=wv[:, sub, h2 * 512 : (h2 + 1) * 512],
                                start=(sub == 0),
                                stop=(sub == 7),
                            )
                    for h2 in range(2):
                        nc.vector.tensor_copy(
                            out=v_out[:, h2 * 512 : (h2 + 1) * 512], in_=accs[h2]
                        )
                    if dst_sb is not None:
                        nc.vector.tensor_copy(
                            out=dst_sb[:, blk].rearrange("p m c -> p (m c)"), in_=v_out
                        )
                    else:
                        b = blk % 4
                        nc.sync.dma_start(
                            out=dst_dram[b // 2, b % 2].rearrange("p m c -> p (m c)"),
                            in_=v_out,
                        )

            # ---- own KT/V -> ktloc/vloc; gather per owned slot asap --------
            def gather(src, dst):
                nc.gpsimd.collective_compute(
                    "AllGather",
                    mybir.AluOpType.bypass,
                    replica_groups=[list(range(NCORES))],
                    ins=[src],
                    outs=[dst],
                )

            ktfulls = []
            for grp in range(2):
                ktfull = projout.tile(
                    [P, 4, 8, P], FP8, tag="ktfull", name=f"ktfull{grp}"
                )
                ktfulls.append(ktfull)
                kt_proj(ktfull, ktloc[grp], wk8_bf, xq8_bf, grp * 512, (grp + 1) * 512)
                gather(ktloc[grp, 0], ktg[2 * grp])
                # interleave kt/v gathers in consumption-deadline order:
                # slot s is consumed before slot s+1, and kt before v
                v_proj(None, vloc[grp], wv1_bf, xq_bf, grp * 512, grp * 512 + 256)
                gather(vloc[grp, 0], vg[2 * grp])
                gather(ktloc[grp, 1], ktg[2 * grp + 1])
                v_proj(None, vloc[grp], wv1_bf, xq_bf, grp * 512 + 256, (grp + 1) * 512)
                gather(vloc[grp, 1], vg[2 * grp + 1])

            # ---- QT (fp8 DoubleRow), stored fp8 at KQ_SCALE ---------------
            for m in range(8):
                accs = [
                    ppsum.tile([P, 512], F32, tag="proj", name=f"qp{n}")
                    for n in range(2)
                ]
                for ss in range(0, 8, 2):
                    for n in range(2):
                        nc.tensor.matmul(
                            accs[n],
                            lhsT=wq8_bf[:, ss : ss + 2, m * P : (m + 1) * P],
                            rhs=xq8_bf[:, ss : ss + 2, n * 512 : (n + 1) * 512],
                            start=(ss == 0),
                            stop=(ss == 6),
                            perf_mode=DR,
                        )
                for n in range(2):
                    nc.scalar.activation(
                        out=qt8_sb[:, m, n * 512 : (n + 1) * 512],
                        in_=accs[n],
                        func=AF.Copy,
                        scale=CAST_SCALE,
                    )

            # gate the masks DMA behind grp0's kt casts so it does not
            # steal wire bandwidth from the critical-path input loads
            nc.vector.tensor_copy(
                out=masks_sb[0:1, 0, 0:1], in_=ktfulls[0][0:1, 0, 0, 0:1]
            )
            nc.scalar.dma_start(out=masks_sb, in_=mask_ext[:])

            # ---- duplicated kv prefix straight into SBUF ------------------
            if b_dup:
                xd_bf = projw.tile([P, 8, b_dup * P], BF16, tag="xd", name="xd")
                nc.scalar.dma_start(out=xd_bf, in_=xd_ext[:])
                xd8_bf = projw.tile([P, 8, b_dup * P], FP8, tag="xd8", name="xd8")
                nc.scalar.activation(
                    out=xd8_bf, in_=xd_bf, func=AF.Copy, scale=X_SCALE
                )
                kt_proj(dup_kt, None, wk8_bf, xd8_bf, 0, b_dup * P)
                v_proj(dup_v, None, wv1_bf, xd_bf, 0, b_dup * P)

        # ---- attention ----------------------------------------------------
        with (
            tc.tile_pool(name="asingles", bufs=1) as asingles,
            tc.tile_pool(name="vpool", bufs=14) as vpool,
            tc.tile_pool(name="epool", bufs=2) as epool,
            tc.tile_pool(name="gpool", bufs=2) as gpool,
            tc.tile_pool(name="ltpsum", bufs=2, space="PSUM") as ltpsum,
            tc.tile_pool(name="hpsum", bufs=1, space="PSUM") as hpsum,
            tc.tile_pool(name="spsum", bufs=1, space="PSUM") as spsum,
            tc.tile_pool(name="tppsum", bufs=1, space="PSUM") as tppsum,
        ):
            wv2_bf = asingles.tile([P, 8, D], BF16, tag="wv2", name="wv2")
            nc.vector.tensor_copy(out=wv2_bf[0:1, 0, 0:1], in_=qt8_sb[0:1, 0, 0:1])
            nc.scalar.dma_start(out=wv2_bf, in_=wv2_ext[:])

            def load_pair(s, j):
                # one DMA each for the (j, j+1) kt / v payloads
                t = j // 2
                s_own = t // 8
                kt2 = vpool.tile([P, 2, 8, P], FP8, tag="kt", name="kt2")
                nc.sync.dma_start(
                    out=kt2,
                    in_=ktg[s_own, t % 8].rearrange("h p m c -> p h m c"),
                )
                v2 = vpool.tile([P, 2, 8, P], BF16, tag="v", name="v2")
                nc.sync.dma_start(
                    out=v2,
                    in_=vg[s_own, t % 8].rearrange("h p m c -> p h m c"),
                )
                return kt2, v2

            def visit_view(s, j, pair):
                if j < b_dup:
                    return dup_kt[:, j], dup_v[:, j].rearrange("p m c -> p (m c)")
                kt2, v2 = pair
                return kt2[:, j % 2], v2[:, j % 2].rearrange("p m c -> p (m c)")

            def logits(s, j, kt_t):
                lt = ltpsum.tile([P, 256], F32, tag="lt", name="lt")
                for ss in range(0, 8, 2):
                    nc.tensor.matmul(
                        lt,
                        lhsT=kt_t[:, ss : ss + 2, :],
                        rhs=qt8_sb[:, ss : ss + 2, s * 256 : (s + 1) * 256],
                        start=(ss == 0),
                        stop=(ss == 6),
                        perf_mode=DR,
                    )
                return lt

            def pv(s, j, lt, v_t, h, sums, jmax):
                pt = vpool.tile([P, 256], BF16, tag="pt", name="pt")
                nc.scalar.activation(out=pt, in_=lt, func=AF.Exp, scale=EXP_SCALE)
                if j >= 16 * s:
                    nc.vector.tensor_mul(out=pt, in0=pt, in1=masks_sb[:, j])
                for qc in range(2):
                    lhsT = pt[:, qc * P : (qc + 1) * P]
                    for dh in range(2):
                        nc.tensor.matmul(
                            h[qc][:, dh, :],
                            lhsT=lhsT,
                            rhs=v_t[:, dh * 512 : (dh + 1) * 512],
                            start=(j == 0),
                            stop=(j == jmax),
                        )
                    nc.tensor.matmul(
                        sums[:, qc : qc + 1],
                        lhsT=lhsT,
                        rhs=ones_sb,
                        start=False,
                        stop=(j == jmax),
                        skip_group_check=True,
                    )

            for s in range(NSLOTS):
                nv = 16 * (s + 1)
                jmax = nv - 1
                h = [
                    hpsum.tile([P, 2, 512], F32, tag=f"hq{qc}", name=f"h{qc}_{s}")
                    for qc in range(2)
                ]
                sums = spsum.tile([P, 2], F32, tag="sums", name="sums")
                nc.tensor.matmul(
                    sums,
                    lhsT=zcol_sb,
                    rhs=zrow_sb[:, :2],
                    start=True,
                    stop=False,
                    skip_group_check=True,
                )
                # software pipeline: logits of j+1 are emitted before pv of j
                pair = None
                prev = None  # (j, lt, v_view)
                for j in range(nv):
                    if j >= b_dup and (j % 2 == 0 or pair is None):
                        pair = load_pair(s, j - (j % 2))
                    kt_t, v_t = visit_view(s, j, pair)
                    lt = logits(s, j, kt_t)
                    if prev is not None:
                        pv(s, prev[0], prev[1], prev[2], h, sums, jmax)
                    prev = (j, lt, v_t)
                pv(s, prev[0], prev[1], prev[2], h, sums, jmax)

                # ---- epilogue ----------------------------------------
                # silu via tanh: silu(u) = (u/2)*(1+tanh(u/2)) — tanh lives
                # in the same activation table as exp, so the per-slot
                # ACT_TABLE_LOAD pair (2.6us serial) disappears entirely
                g_bf = []
                for qc in range(2):
                    recip = epool.tile([P, 1], F32, tag="recip", name="recip")
                    nc.vector.reciprocal(out=recip, in_=sums[:, qc : qc + 1])
                    recip2 = epool.tile([P, 1], F32, tag="recip2", name="recip2")
                    nc.vector.tensor_scalar_mul(out=recip2, in0=recip, scalar1=0.5)
                    th = gpool.tile([P, 1024], BF16, tag=f"th{qc}", name=f"th{qc}")
                    nc.scalar.activation(
                        out=th,
                        in_=h[qc].rearrange("p a b -> p (a b)"),
                        func=AF.Tanh,
                        scale=recip2,
                    )
                    uh = gpool.tile([P, 2, 512], BF16, tag=f"uh{qc}", name=f"uh{qc}")
                    nc.vector.tensor_scalar_mul(out=uh, in0=h[qc], scalar1=recip2)
                    tp1 = gpool.tile([P, 1024], BF16, tag=f"tp{qc}", name=f"tp{qc}")
                    nc.vector.tensor_scalar_add(out=tp1, in0=th, scalar1=1.0)
                    g = gpool.tile([P, 1024], BF16, tag=f"g{qc}", name=f"g{qc}")
                    nc.vector.tensor_mul(
                        out=g, in0=uh.rearrange("p a b -> p (a b)"), in1=tp1
                    )
                    g_bf.append(g)
                # transpose G -> gt [d-part, m, 256]
                gt_sb = epool.tile([P, 8, 256], BF16, tag="gt", name="gt")
                for m in range(8):
                    for qc in range(2):
                        tp = tppsum.tile([P, 256], BF16, tag="tp", name="tp")
                        nc.tensor.transpose(
                            tp[:, :P],
                            g_bf[qc][:, m * P : (m + 1) * P],
                            ident_sb,
                        )
                        nc.vector.tensor_copy(
                            out=gt_sb[:, m, qc * P : (qc + 1) * P], in_=tp[:, :P]
                        )
                # output projection: O[q, d] via lhsT = gt chunks
                for qc in range(2):
                    op = hpsum.tile(
                        [P, 2, 512], F32, tag=f"hq{qc}", name=f"o{qc}_{s}"
                    )
                    for m in range(8):
                        for dh in range(2):
                            nc.tensor.matmul(
                                op[:, dh, :],
                                lhsT=gt_sb[:, m, qc * P : (qc + 1) * P],
                                rhs=wv2_bf[:, m, dh * 512 : (dh + 1) * 512],
                                start=(m == 0),
                                stop=(m == 7),
                            )
                    oo = epool.tile([P, 2, 512], BF16, tag="oo", name="oo")
                    nc.vector.tensor_copy(out=oo, in_=op)
                    nc.scalar.dma_start(
                        out=o_ext[s, qc], in_=oo.rearrange("p a b -> p (a b)")
                    )

        singles_ctx.__exit__(None, None, None)

    nc.finalize()
    return nc


_NC_CACHE = {}


def get_nc(b_dup=B_DUP):
    if b_dup not in _NC_CACHE:
        _NC_CACHE[b_dup] = build_kernel(b_dup)
    return _NC_CACHE[b_dup]


def build_masks():
    """Masks for the last 16 visits of each slot, selected per core by
    k = 2c + 16s - j: k>=1 all-visible, k==0 upper-left triangle, k==-1
    shifted triangle, k<=-2 fully masked (padded visit)."""
    p = np.arange(P)[:, None]
    u = np.arange(256)[None, :]
    m_ones = np.ones((P, 256), np.float32)
    m0 = (p <= u).astype(np.float32)
    m1 = (p <= u - P).astype(np.float32)
    m_zero = np.zeros((P, 256), np.float32)
    canon = np.stack([m_zero, m1, m0, m_ones]).astype(ml_dtypes.float8_e4m3)

    out = []
    for c in range(NCORES):
        sel = []
        for s in range(NSLOTS):
            for j in range(16 * s, 16 * (s + 1)):
                k = 2 * c + 16 * s - j
                sel.append(min(max(k, -2), 1) + 2)
        out.append(canon[np.array(sel, np.int64)])
    return out  # list of [64, 128, 256] bf16


def _to_fp8(x):
    return np.ascontiguousarray(
        np.clip(x, -240.0, 240.0).astype(ml_dtypes.float8_e4m3)
    )


def _arr(a):
    """[D, N] -> [P, 8, N] with d = sub*128 + p."""
    return np.ascontiguousarray(a.reshape(8, P, -1).transpose(1, 0, 2))


def build_in_maps(x, wq, wk, wv1, wv2, b_dup=B_DUP):
    bf = ml_dtypes.bfloat16
    xT = np.ascontiguousarray(np.asarray(x, np.float32).T)
    masks = build_masks()
    xd32 = np.ascontiguousarray(xT[:, : b_dup * P])
    w = {
        "wq8": _to_fp8(_arr(np.asarray(wq, np.float32) * W_SCALE)),
        "wk8": _to_fp8(_arr(np.asarray(wk, np.float32) * W_SCALE)),
        "wv1": np.ascontiguousarray(_arr(np.asarray(wv1, np.float32)).astype(bf)),
        "wv2": np.ascontiguousarray(_arr(np.asarray(wv2, np.float32)).astype(bf)),
    }
    if b_dup:
        w["xd"] = np.ascontiguousarray(_arr(xd32).astype(bf))
    in_maps = []
    for c in range(NCORES):
        xq_c = np.concatenate(
            [xT[:, 256 * (c + 8 * s) : 256 * (c + 8 * s) + 256] for s in range(NSLOTS)],
            axis=1,
        )
        in_maps.append(
            {
                "xq8": _to_fp8(_arr(xq_c * X_SCALE)),
                "xq": np.ascontiguousarray(_arr(xq_c).astype(bf)),
                "masks": np.ascontiguousarray(masks[c].transpose(1, 0, 2)),
                **w,
            }
        )
    return in_maps


def assemble_out(results):
    out = np.empty((SEQ, D), np.float32)
    for c in range(NCORES):
        o = results[c]["o"]  # [4, 2, 128, 1024]
        for s in range(NSLOTS):
            r0 = 256 * (c + 8 * s)
            out[r0 : r0 + P, :] = np.asarray(o[s, 0], np.float32)
            out[r0 + P : r0 + 256, :] = np.asarray(o[s, 1], np.float32)
    return out


def kernel(x, wq, wk, wv1, wv2):
    in_maps = build_in_maps(x, wq, wk, wv1, wv2)
    nc = get_nc()
    res = run_bass_kernel_spmd(nc, in_maps, list(range(NCORES)))
    return assemble_out(res.results)
